# revision 4
# baseline (speedup 1.0000x reference)
"""Batch-parallel attention kernel for 8 TRN2 NeuronCores.

Problem: B=16, S=2048, D=128 full (non-causal) attention, fp32 I/O.
Sharding: batch dim across 8 cores (2 batches/core), no collectives.

Per-core layout trick: everything is computed in "transposed score" space
S^T[k, q] so that no on-device transposes are needed:
  - matmul1: S^T[k,q] = (K^T)[d,k]^T-stationary @ (Q^T)[d,q]-moving,
    contraction over d=128 partitions. Q^T/K^T are prepared on host.
  - softmax numerator exp(scale*S^T) is split 75/25 between two engines
    so the ScalarE exp stream (the old pacer at ~1 elem/cycle/lane) drops
    below the TensorE per-group work:
      * q-columns 0:384 of each 512-wide chunk: ScalarE table exp,
        PSUM->SBUF bf16 (no max subtraction; scores ~N(0,1), max ~7.5).
      * q-columns 384:512: VectorE Schraudolph exp - one tensor_scalar
        (x*A+B -> int32); the int32 bit pattern IS the fp32 of
        2^(x*log2e) up to a +-3% PWL ripple, and matmul2 reads the high
        16 bits of each int32 as bf16 (bf16 == truncated fp32) through a
        stride-2 bitcast view. Zero extra elementwise ops. B includes
        +0x8000 so the bf16 truncation rounds to nearest.
  - matmul2: out[q, 0:129] = sum_k expS^T[k,q]^T-stationary @ V_aug[k,:]
    where V_aug = [V | ones]; column 128 accumulates the softmax
    denominator exactly in fp32 PSUM (consistent with the approximated
    numerator, so common-mode exp error cancels).
  - normalize: VectorE reciprocal of the denominator + GpSimd
    tensor_scalar multiply (keeps VectorE under the TensorE pace).

Steady state is paced by TensorE at ~880ns per 2-ktile group
(2x215ns m1 + 8x56ns m2, both at ideal issue rates). First-tile input
DMAs are spread across four idle rings to pull the first matmul in.

PSUM budget: 4 banks score groups (2x [128,2,512]) + accumulators.
"""

import math
import os

import ml_dtypes
import numpy as np

import concourse.bass as bass
import concourse.mybir as mybir
import concourse.tile as tile
from concourse import bacc
from concourse.bass_utils import run_bass_kernel_spmd

B, S, D = 16, 2048, 128
N_CORES = 8
BPC = B // N_CORES          # batches per core
DA = D + 1                  # V augmented with ones column
QCHUNK = 512                # q processed per inner pipeline chunk
N_QC = S // QCHUNK          # 4
N_KT = S // 128             # 16 k-tiles
KT_GROUPS = [(k, 2) for k in range(0, 16, 2)]
SCALE = 1.0 / float(np.sqrt(D))

Q_EXP = 384                 # q-cols per chunk on ScalarE true exp
Q_SCH = QCHUNK - Q_EXP      # q-cols per chunk on VectorE Schraudolph

# Schraudolph exp: bitcast(int32(x*SCH_A + SCH_B)) ~= exp(x*SCALE).
# A = 2^23 * log2(e) * SCALE; B centers the PWL ripple multiplicatively
# (shift by log2(r*)/2, r* = max (1+f)/2^f) and pre-adds 0x8000 so the
# later bf16 truncation is round-to-nearest.
_L = float(2 ** 23)
_RSTAR = (1.0 / math.log(2.0)) / 2.0 ** (1.0 / math.log(2.0) - 1.0)
SCH_A = _L * math.log2(math.e) * SCALE
SCH_B = 127.0 * _L - math.log2(_RSTAR) / 2.0 * _L + 32768.0

BF16 = mybir.dt.bfloat16
F32 = mybir.dt.float32
I32 = mybir.dt.int32

TRACE = bool(os.environ.get("BASS_KERNEL_TRACE"))
LAST_RESULTS = None

_CACHE = {}


def _build():
    nc = bacc.Bacc("TRN2", target_bir_lowering=False, debug=False)

    qT = nc.dram_tensor("qT", [BPC, D, S], BF16, kind="ExternalInput").ap()
    kT = nc.dram_tensor("kT", [BPC, D, S], BF16, kind="ExternalInput").ap()
    vA = nc.dram_tensor("vA", [BPC, S, DA], BF16, kind="ExternalInput").ap()
    out = nc.dram_tensor("out", [BPC, S, D], F32, kind="ExternalOutput").ap()

    with tile.TileContext(nc) as tc:
        with (
            tc.tile_pool(name="qk", bufs=2) as qk_pool,
            tc.tile_pool(name="vp", bufs=2) as v_pool,
            tc.tile_pool(name="warm", bufs=1) as warm_pool,
            tc.tile_pool(name="pexp", bufs=6) as p_pool,
            tc.tile_pool(name="psch", bufs=6) as p32_pool,
            tc.tile_pool(name="outs", bufs=12) as o_pool,
            tc.tile_pool(name="psum_s", bufs=1, space="PSUM") as psum_s,
            tc.tile_pool(name="psum_acc", bufs=1, space="PSUM") as psum_acc,
        ):
            # Pull the ~2.7us exp table load to t=0 so it overlaps the input
            # DMAs instead of stalling the first real exp.
            wtile = warm_pool.tile([128, 1], F32)
            nc.vector.memset(wtile, 0.0)
            nc.scalar.activation(
                wtile, wtile, mybir.ActivationFunctionType.Exp
            )

            QS = S // 4
            batch_tiles = {}

            def load_batch(b):
                kT_sb = [qk_pool.tile([128, QS], BF16, tag=f"kT{h}",
                                      name=f"kT{h}") for h in range(4)]
                qT_sb = [qk_pool.tile([128, QS], BF16, tag=f"qT{h}",
                                      name=f"qT{h}") for h in range(4)]
                v_sb = [v_pool.tile([128, N_KT // 2, DA], BF16, tag=f"v{h}",
                                    name=f"v{h}") for h in range(2)]
                v0 = vA[b][0 : S // 2].rearrange("(t p) d -> p t d", p=128)
                v1 = vA[b][S // 2 : S].rearrange("(t p) d -> p t d", p=128)
                if b == 0:
                    # Latency-critical first tiles: the first m1 needs
                    # kT[:,0:256] + qT[:,0:512]; land them on the three
                    # DMA-capable rings (sync/scalar/gpsimd) in parallel.
                    nc.sync.dma_start(out=kT_sb[0][:, 0:256],
                                      in_=kT[b][:, 0:256])
                    nc.scalar.dma_start(out=qT_sb[0][:, 0:256],
                                        in_=qT[b][:, 0:256])
                    nc.scalar.dma_start(out=qT_sb[0][:, 256:512],
                                        in_=qT[b][:, 256:512])
                    nc.gpsimd.dma_start(out=kT_sb[0][:, 256:512],
                                        in_=kT[b][:, 256:512])
                    nc.sync.dma_start(out=kT_sb[1], in_=kT[b][:, QS : 2 * QS])
                    nc.gpsimd.dma_start(out=v_sb[0], in_=v0)
                    nc.sync.dma_start(out=kT_sb[2],
                                      in_=kT[b][:, 2 * QS : 3 * QS])
                    nc.gpsimd.dma_start(out=qT_sb[1],
                                        in_=qT[b][:, QS : 2 * QS])
                    nc.sync.dma_start(out=kT_sb[3], in_=kT[b][:, 3 * QS : S])
                    nc.gpsimd.dma_start(out=v_sb[1], in_=v1)
                    nc.sync.dma_start(out=qT_sb[2],
                                      in_=qT[b][:, 2 * QS : 3 * QS])
                    nc.gpsimd.dma_start(out=qT_sb[3],
                                        in_=qT[b][:, 3 * QS : S])
                else:
                    # Mid-stream prefetch: only the sync + gpsimd rings
                    # (scalar/vector/tensor queues are saturated).
                    nc.sync.dma_start(out=kT_sb[0], in_=kT[b][:, 0:QS])
                    nc.gpsimd.dma_start(out=qT_sb[0], in_=qT[b][:, 0:QS])
                    nc.sync.dma_start(out=kT_sb[1], in_=kT[b][:, QS : 2 * QS])
                    nc.gpsimd.dma_start(out=v_sb[0], in_=v0)
                    nc.sync.dma_start(out=kT_sb[2],
                                      in_=kT[b][:, 2 * QS : 3 * QS])
                    nc.gpsimd.dma_start(out=qT_sb[1],
                                        in_=qT[b][:, QS : 2 * QS])
                    nc.sync.dma_start(out=kT_sb[3], in_=kT[b][:, 3 * QS : S])
                    nc.gpsimd.dma_start(out=qT_sb[2],
                                        in_=qT[b][:, 2 * QS : 3 * QS])
                    nc.sync.dma_start(out=qT_sb[3], in_=qT[b][:, 3 * QS : S])
                    nc.gpsimd.dma_start(out=v_sb[1], in_=v1)
                batch_tiles[b] = (kT_sb, qT_sb, v_sb)

            def emit_m2(b, qc, kt0, n_kt, p_tile, p32_t, acc):
                _, _, v_sb = batch_tiles[b]
                # int32 Schraudolph words: high 16 bits of each are the
                # bf16 of the approximated exp.
                p32_bf = p32_t[:, :, :].bitcast(BF16)
                for h in range(n_kt):
                    kt = kt0 + h
                    for j in range(4):
                        if j < 3:
                            lhsT = p_tile[:, h, j * 128 : (j + 1) * 128]
                        else:
                            lhsT = p32_bf[:, h, 1::2]
                        # start=True clears has_written for the WHOLE bank,
                        # so only the first slice of each packed bank may
                        # carry it; the second slice's first write lands on
                        # cleared bits and overwrites.
                        nc.tensor.matmul(
                            acc[j],
                            lhsT=lhsT,
                            rhs=v_sb[kt // 8][:, kt % 8, :],
                            start=(kt == 0),
                            stop=(kt == N_KT - 1),
                        )
                if kt0 + n_kt == N_KT:
                    emit_normalize(b, qc, acc)

            def emit_normalize(b, qc, acc):
                for j in range(4):
                    a = acc[j]
                    recip = o_pool.tile([128, 1], F32, tag="recip",
                                        name="recip")
                    nc.vector.reciprocal(recip, a[:, D : D + 1])
                    o_sb = o_pool.tile([128, D], F32, tag="o", name="o_sb")
                    nc.vector.tensor_scalar_mul(o_sb, a[:, 0:D], recip)
                    r0 = qc * QCHUNK + j * 128
                    eng = nc.sync if j % 2 == 0 else nc.gpsimd
                    eng.dma_start(out=out[b, r0 : r0 + 128, :], in_=o_sb)

            # One continuous software pipeline across every (batch, q-chunk,
            # k-group): m2 for group g is emitted after m1 of group g+2, so
            # the in-order PE queue always has independent m1 work while exp
            # runs, with no pipeline drain at q-chunk or batch boundaries.
            pending = []
            load_batch(0)
            for b in range(BPC):
                for qc in range(N_QC):
                    kT_sb, qT_sb, _ = batch_tiles[b]
                    acc = [
                        psum_acc.tile(
                            [128, DA], F32, tag=f"acc{j}", name=f"acc{j}"
                        )
                        for j in range(4)
                    ]
                    for kt0, n_kt in KT_GROUPS:
                        ab = "A" if (kt0 // 2) % 2 == 0 else "B"
                        s_psum = psum_s.tile(
                            [128, n_kt, QCHUNK], F32, tag=f"s{ab}",
                            name=f"s{ab}",
                        )
                        for h in range(n_kt):
                            kt = kt0 + h
                            nc.tensor.matmul(
                                s_psum[:, h, :],
                                lhsT=kT_sb[kt // 4][
                                    :, (kt % 4) * 128 : (kt % 4 + 1) * 128
                                ],
                                rhs=qT_sb[qc],
                                start=True,
                                stop=True,
                            )
                        p_tile = p_pool.tile(
                            [128, n_kt, Q_EXP], BF16, tag=f"p{ab}",
                            name=f"p{ab}",
                        )
                        nc.scalar.activation(
                            p_tile,
                            s_psum[:, :, 0:Q_EXP],
                            mybir.ActivationFunctionType.Exp,
                            scale=SCALE,
                        )
                        p32_t = p32_pool.tile(
                            [128, n_kt, Q_SCH], I32, tag=f"g{ab}",
                            name=f"g{ab}",
                        )
                        nc.vector.tensor_scalar(
                            p32_t,
                            s_psum[:, :, Q_EXP:QCHUNK],
                            SCH_A,
                            SCH_B,
                            op0=mybir.AluOpType.mult,
                            op1=mybir.AluOpType.add,
                        )
                        pending.append((b, qc, kt0, n_kt, p_tile, p32_t, acc))
                        if len(pending) > 3:
                            emit_m2(*pending.pop(0))
                        # prefetch next batch's inputs once this batch's
                        # first q-chunk is underway
                        if b + 1 < BPC and qc == 1 and kt0 == 6:
                            load_batch(b + 1)
            for args in pending:
                emit_m2(*args)

    nc.compile()
    return nc


def _get_nc():
    if "nc" not in _CACHE:
        _CACHE["nc"] = _build()
    return _CACHE["nc"]


def kernel(query, key, value):
    global LAST_RESULTS
    bf16 = ml_dtypes.bfloat16
    q = np.ascontiguousarray(
        np.asarray(query, dtype=np.float32).transpose(0, 2, 1)
    ).astype(bf16)
    k = np.ascontiguousarray(
        np.asarray(key, dtype=np.float32).transpose(0, 2, 1)
    ).astype(bf16)
    v = np.asarray(value, dtype=np.float32)
    v_aug = np.concatenate(
        [v, np.ones((B, S, 1), dtype=np.float32)], axis=2
    ).astype(bf16)

    nc = _get_nc()
    in_maps = [
        {
            "qT": q[i * BPC : (i + 1) * BPC],
            "kT": k[i * BPC : (i + 1) * BPC],
            "vA": v_aug[i * BPC : (i + 1) * BPC],
        }
        for i in range(N_CORES)
    ]
    res = run_bass_kernel_spmd(
        nc, in_maps, core_ids=list(range(N_CORES)), trace=TRACE
    )
    LAST_RESULTS = res
    out = np.empty((B, S, D), dtype=np.float32)
    for i in range(N_CORES):
        out[i * BPC : (i + 1) * BPC] = res.results[i]["out"]
    return out


# revision 7
# speedup vs baseline: 1.0822x; 1.0822x over previous
"""Batch-parallel attention kernel for 8 TRN2 NeuronCores.

Problem: B=16, S=2048, D=128 full (non-causal) attention, fp32 I/O.
Sharding: batch dim across 8 cores (2 batches/core), no collectives.

Per-core layout trick: everything is computed in "transposed score" space
S^T[k, q] so that no on-device transposes are needed:
  - matmul1: S^T[k,q] = (K^T)[d,k]^T-stationary @ (Q^T)[d,q]-moving,
    contraction over d=128 partitions. Q^T/K^T are prepared on host.
  - softmax numerator exp(scale*S^T) is split 75/25 between two engines
    so the ScalarE exp stream (the old pacer at ~1 elem/cycle/lane) drops
    below the TensorE per-group work:
      * q-columns 0:384 of each 512-wide chunk: ScalarE table exp,
        PSUM->SBUF bf16 (no max subtraction; scores ~N(0,1), max ~7.5).
      * q-columns 384:512: VectorE Schraudolph exp - one tensor_scalar
        (x*A+B -> int32); the int32 bit pattern IS the fp32 of
        2^(x*log2e) up to a +-3% PWL ripple, and matmul2 reads the high
        16 bits of each int32 as bf16 (bf16 == truncated fp32) through a
        stride-2 bitcast view. Zero extra elementwise ops. B includes
        +0x8000 so the bf16 truncation rounds to nearest.
  - matmul2: out[q, 0:129] = sum_k expS^T[k,q]^T-stationary @ V_aug[k,:]
    where V_aug = [V | ones]; column 128 accumulates the softmax
    denominator exactly in fp32 PSUM (consistent with the approximated
    numerator, so common-mode exp error cancels).
  - normalize: VectorE reciprocal + tensor_scalar multiply. Jobs are
    deferred and drained ONE PER K-GROUP so they never lump on the DVE
    queue between Schraudolph ops (a lump delays the score-PSUM-free
    signal and stalls m1). The final q-chunk normalizes per-j inline
    right behind its last matmuls to shorten the drain tail.

Steady state is paced by TensorE at ~880ns per 2-ktile group
(2x215ns m1 + 8x56ns m2 at ideal issue rates). Score PSUM cycles
through 3 groups (A/B/C) so m1 of group g only needs bank g-3 free.
Startup: input rings are ordered so the first-needed 256KB heads each
DMA ring, and the first k-group's m1s are split into 256-column halves
to start on the first landed qT half.

PSUM budget: 6 banks score groups (3x [128,2,512]) + 2 banks accs.
"""

import math
import os

import ml_dtypes
import numpy as np

import concourse.bass as bass
import concourse.mybir as mybir
import concourse.tile as tile
from concourse import bacc
from concourse.bass_utils import run_bass_kernel_spmd

B, S, D = 16, 2048, 128
N_CORES = 8
BPC = B // N_CORES          # batches per core
DA = D + 1                  # V augmented with ones column
QCHUNK = 512                # q processed per inner pipeline chunk
N_QC = S // QCHUNK          # 4
N_KT = S // 128             # 16 k-tiles
KT_GROUPS = [(k, 2) for k in range(0, 16, 2)]
SCALE = 1.0 / float(np.sqrt(D))

Q_EXP = 384                 # q-cols per chunk on ScalarE true exp
Q_SCH = QCHUNK - Q_EXP      # q-cols per chunk on VectorE Schraudolph

# Schraudolph exp: bitcast(int32(x*SCH_A + SCH_B)) ~= exp(x*SCALE).
# A = 2^23 * log2(e) * SCALE; B centers the PWL ripple multiplicatively
# (shift by log2(r*)/2, r* = max (1+f)/2^f) and pre-adds 0x8000 so the
# later bf16 truncation is round-to-nearest.
_L = float(2 ** 23)
_RSTAR = (1.0 / math.log(2.0)) / 2.0 ** (1.0 / math.log(2.0) - 1.0)
SCH_A = _L * math.log2(math.e) * SCALE
SCH_B = 127.0 * _L - math.log2(_RSTAR) / 2.0 * _L + 32768.0

BF16 = mybir.dt.bfloat16
F32 = mybir.dt.float32
I32 = mybir.dt.int32

TRACE = bool(os.environ.get("BASS_KERNEL_TRACE"))
LAST_RESULTS = None

_CACHE = {}


def _build():
    nc = bacc.Bacc("TRN2", target_bir_lowering=False, debug=False)

    qT = nc.dram_tensor("qT", [BPC, D, S], BF16, kind="ExternalInput").ap()
    kT = nc.dram_tensor("kT", [BPC, D, S], BF16, kind="ExternalInput").ap()
    vA = nc.dram_tensor("vA", [BPC, S, DA], BF16, kind="ExternalInput").ap()
    out = nc.dram_tensor("out", [BPC, S, D], F32, kind="ExternalOutput").ap()

    with tile.TileContext(nc) as tc:
        with (
            tc.tile_pool(name="qk", bufs=2) as qk_pool,
            tc.tile_pool(name="vp", bufs=2) as v_pool,
            tc.tile_pool(name="warm", bufs=1) as warm_pool,
            tc.tile_pool(name="pexp", bufs=6) as p_pool,
            tc.tile_pool(name="psch", bufs=6) as p32_pool,
            tc.tile_pool(name="outs", bufs=12) as o_pool,
            tc.tile_pool(name="psum_s", bufs=1, space="PSUM") as psum_s,
            tc.tile_pool(name="psum_acc", bufs=1, space="PSUM") as psum_acc,
        ):
            # Pull the ~2.7us exp table load to t=0 so it overlaps the input
            # DMAs instead of stalling the first real exp.
            wtile = warm_pool.tile([128, 1], F32)
            nc.vector.memset(wtile, 0.0)
            nc.scalar.activation(
                wtile, wtile, mybir.ActivationFunctionType.Exp
            )

            QS = S // 4
            batch_tiles = {}

            def load_batch(b):
                kT_sb = [qk_pool.tile([128, QS], BF16, tag=f"kT{h}",
                                      name=f"kT{h}") for h in range(4)]
                qT_sb = [qk_pool.tile([128, QS], BF16, tag=f"qT{h}",
                                      name=f"qT{h}") for h in range(4)]
                v_sb = [v_pool.tile([128, N_KT // 2, DA], BF16, tag=f"v{h}",
                                    name=f"v{h}") for h in range(2)]
                v0 = vA[b][0 : S // 2].rearrange("(t p) d -> p t d", p=128)
                v1 = vA[b][S // 2 : S].rearrange("(t p) d -> p t d", p=128)
                if b == 0:
                    # A ring drains its descriptors in order, so each ring
                    # leads with the tiles the first k-groups need; the
                    # three rings then share HBM bandwidth ~fairly.
                    nc.sync.dma_start(out=kT_sb[0][:, 0:256],
                                      in_=kT[b][:, 0:256])
                    nc.scalar.dma_start(out=qT_sb[0][:, 0:256],
                                        in_=qT[b][:, 0:256])
                    nc.scalar.dma_start(out=qT_sb[0][:, 256:512],
                                        in_=qT[b][:, 256:512])
                    nc.sync.dma_start(out=kT_sb[0][:, 256:512],
                                      in_=kT[b][:, 256:512])
                    nc.sync.dma_start(out=kT_sb[1], in_=kT[b][:, QS : 2 * QS])
                    nc.gpsimd.dma_start(out=v_sb[0], in_=v0)
                    nc.sync.dma_start(out=kT_sb[2],
                                      in_=kT[b][:, 2 * QS : 3 * QS])
                    nc.sync.dma_start(out=kT_sb[3], in_=kT[b][:, 3 * QS : S])
                    nc.gpsimd.dma_start(out=qT_sb[1],
                                        in_=qT[b][:, QS : 2 * QS])
                    nc.gpsimd.dma_start(out=v_sb[1], in_=v1)
                    nc.gpsimd.dma_start(out=qT_sb[2],
                                        in_=qT[b][:, 2 * QS : 3 * QS])
                    nc.gpsimd.dma_start(out=qT_sb[3],
                                        in_=qT[b][:, 3 * QS : S])
                else:
                    # Mid-stream prefetch: only the sync + gpsimd rings
                    # (scalar/vector queues are saturated).
                    nc.sync.dma_start(out=kT_sb[0], in_=kT[b][:, 0:QS])
                    nc.gpsimd.dma_start(out=qT_sb[0], in_=qT[b][:, 0:QS])
                    nc.sync.dma_start(out=kT_sb[1], in_=kT[b][:, QS : 2 * QS])
                    nc.gpsimd.dma_start(out=v_sb[0], in_=v0)
                    nc.sync.dma_start(out=kT_sb[2],
                                      in_=kT[b][:, 2 * QS : 3 * QS])
                    nc.gpsimd.dma_start(out=qT_sb[1],
                                        in_=qT[b][:, QS : 2 * QS])
                    nc.sync.dma_start(out=kT_sb[3], in_=kT[b][:, 3 * QS : S])
                    nc.gpsimd.dma_start(out=qT_sb[2],
                                        in_=qT[b][:, 2 * QS : 3 * QS])
                    nc.sync.dma_start(out=qT_sb[3], in_=qT[b][:, 3 * QS : S])
                    nc.gpsimd.dma_start(out=v_sb[1], in_=v1)
                batch_tiles[b] = (kT_sb, qT_sb, v_sb)

            norm_queue = []

            def emit_norm_job(b, qc, acc, j, eng=None):
                a = acc[j]
                recip = o_pool.tile([128, 1], F32, tag="recip",
                                    name="recip")
                nc.vector.reciprocal(recip, a[:, D : D + 1])
                o_sb = o_pool.tile([128, D], F32, tag="o", name="o_sb")
                nc.vector.tensor_scalar_mul(o_sb, a[:, 0:D], recip)
                r0 = qc * QCHUNK + j * 128
                if eng is None:
                    eng = nc.sync if j % 2 == 0 else nc.gpsimd
                eng.dma_start(out=out[b, r0 : r0 + 128, :], in_=o_sb)

            def emit_m2(b, qc, kt0, n_kt, p_tile, p32_t, acc):
                _, _, v_sb = batch_tiles[b]
                # int32 Schraudolph words: high 16 bits of each are the
                # bf16 of the approximated exp.
                p32_bf = p32_t[:, :, :].bitcast(BF16)
                final = (b == BPC - 1 and qc == N_QC - 1
                         and kt0 + n_kt == N_KT)
                # Issue the last-qc out DMAs on three different rings so
                # their ~600ns descriptor setups don't serialize the tail.
                tail_engs = [nc.sync, nc.scalar, nc.gpsimd, nc.sync]
                for h in range(n_kt):
                    kt = kt0 + h
                    for j in range(4):
                        if j < 3:
                            lhsT = p_tile[:, h, j * 128 : (j + 1) * 128]
                        else:
                            lhsT = p32_bf[:, h, 1::2]
                        # Accumulators are packed two per PSUM bank.
                        # start=True clears has_written for the WHOLE bank,
                        # so only the first slice of each packed bank may
                        # carry it; the second slice's first write lands on
                        # cleared bits and overwrites.
                        nc.tensor.matmul(
                            acc[j],
                            lhsT=lhsT,
                            rhs=v_sb[kt // 8][:, kt % 8, :],
                            start=(kt == 0 and j % 2 == 0),
                            stop=(kt == N_KT - 1),
                        )
                        if final and kt == N_KT - 1:
                            emit_norm_job(b, qc, acc, j, eng=tail_engs[j])
                if kt0 + n_kt == N_KT and not final:
                    for j in range(4):
                        norm_queue.append((b, qc, acc, j))

            # One continuous software pipeline across every (batch, q-chunk,
            # k-group): m2 for group g is emitted after m1 of group g+2, so
            # the in-order PE queue always has independent m1 work while exp
            # runs, with no pipeline drain at q-chunk or batch boundaries.
            pending = []
            load_batch(0)
            first_group = True
            for b in range(BPC):
                for qc in range(N_QC):
                    kT_sb, qT_sb, _ = batch_tiles[b]
                    acc_t = [
                        psum_acc.tile(
                            [128, 2, DA], F32, tag=f"acc{i}", name=f"acc{i}"
                        )
                        for i in range(2)
                    ]
                    acc = [acc_t[j // 2][:, j % 2, :] for j in range(4)]
                    for kt0, n_kt in KT_GROUPS:
                        ab = "ABC"[(kt0 // 2) % 3]
                        s_psum = psum_s.tile(
                            [128, n_kt, QCHUNK], F32, tag=f"s{ab}",
                            name=f"s{ab}",
                        )
                        for h in range(n_kt):
                            kt = kt0 + h
                            lhsT = kT_sb[kt // 4][
                                :, (kt % 4) * 128 : (kt % 4 + 1) * 128
                            ]
                            if first_group:
                                # split into 256-col halves so the first
                                # matmul starts on the first landed qT half
                                for c0 in (0, 256):
                                    nc.tensor.matmul(
                                        s_psum[:, h, c0 : c0 + 256],
                                        lhsT=lhsT,
                                        rhs=qT_sb[qc][:, c0 : c0 + 256],
                                        start=True,
                                        stop=True,
                                    )
                            else:
                                nc.tensor.matmul(
                                    s_psum[:, h, :],
                                    lhsT=lhsT,
                                    rhs=qT_sb[qc],
                                    start=True,
                                    stop=True,
                                )
                        first_group = False
                        p_tile = p_pool.tile(
                            [128, n_kt, Q_EXP], BF16, tag=f"p{ab}",
                            name=f"p{ab}",
                        )
                        nc.scalar.activation(
                            p_tile,
                            s_psum[:, :, 0:Q_EXP],
                            mybir.ActivationFunctionType.Exp,
                            scale=SCALE,
                        )
                        p32_t = p32_pool.tile(
                            [128, n_kt, Q_SCH], I32, tag=f"g{ab}",
                            name=f"g{ab}",
                        )
                        nc.vector.tensor_scalar(
                            p32_t,
                            s_psum[:, :, Q_EXP:QCHUNK],
                            SCH_A,
                            SCH_B,
                            op0=mybir.AluOpType.mult,
                            op1=mybir.AluOpType.add,
                        )
                        pending.append((b, qc, kt0, n_kt, p_tile, p32_t, acc))
                        if len(pending) > 3:
                            emit_m2(*pending.pop(0))
                        if norm_queue:
                            emit_norm_job(*norm_queue.pop(0))
                        # prefetch next batch's inputs once this batch's
                        # first q-chunk is underway
                        if b + 1 < BPC and qc == 1 and kt0 == 6:
                            load_batch(b + 1)
            for args in pending:
                emit_m2(*args)
                if norm_queue:
                    emit_norm_job(*norm_queue.pop(0))
            while norm_queue:
                emit_norm_job(*norm_queue.pop(0))

    nc.compile()
    return nc


def _get_nc():
    if "nc" not in _CACHE:
        _CACHE["nc"] = _build()
    return _CACHE["nc"]


def kernel(query, key, value):
    global LAST_RESULTS
    bf16 = ml_dtypes.bfloat16
    q = np.ascontiguousarray(
        np.asarray(query, dtype=np.float32).transpose(0, 2, 1)
    ).astype(bf16)
    k = np.ascontiguousarray(
        np.asarray(key, dtype=np.float32).transpose(0, 2, 1)
    ).astype(bf16)
    v = np.asarray(value, dtype=np.float32)
    v_aug = np.concatenate(
        [v, np.ones((B, S, 1), dtype=np.float32)], axis=2
    ).astype(bf16)

    nc = _get_nc()
    in_maps = [
        {
            "qT": q[i * BPC : (i + 1) * BPC],
            "kT": k[i * BPC : (i + 1) * BPC],
            "vA": v_aug[i * BPC : (i + 1) * BPC],
        }
        for i in range(N_CORES)
    ]
    res = run_bass_kernel_spmd(
        nc, in_maps, core_ids=list(range(N_CORES)), trace=TRACE
    )
    LAST_RESULTS = res
    out = np.empty((B, S, D), dtype=np.float32)
    for i in range(N_CORES):
        out[i * BPC : (i + 1) * BPC] = res.results[i]["out"]
    return out
